# revision 12
# baseline (speedup 1.0000x reference)
"""Trainium2 Bass kernel for nn_DecoderAttention (B=2, L=1024, D=2048, H=16).

Sharding: tensor-parallel over heads (2 heads / core, 8 cores).  v2: all
projection/AV matmuls run in fp8 e4m3 with MatmulPerfMode.DoubleRow
(K=256 per instruction, 2x bf16 throughput); attention scores stay bf16.

Per core:
  1. Q/K projection for its 2 heads (fp8 DoubleRow, i-outer "chase"
     ordering so the PE starts as soon as the first 2 D-chunks land).
     RoPE applied with a host-side NeoX (even/odd) row permutation of
     Wq/Wk so rotation is elementwise on 64-partition halves.
  2. V in token-major layout (fp8 DoubleRow), drained to fp8 SBUF.
  3. Attention per (batch, head): scores^T = K @ Q^T in bf16, exp(s-2)
     on ScalarE straight to fp8 (shift keeps exp under the e4m3 max),
     softmax denominators via f16 tree-add + ones-matmul, out^T via
     fp8 DoubleRow with tok-major V stationary.
  4. Per-(batch,head) AllToAll of the normalized fp8 out^T so core c
     ends with the full 2048 head-dims of its 256 tokens; fp8 DoubleRow
     output projection chases the arriving halves; residual + LayerNorm.

Numerics: Wq/Wk/Wv/Wo shipped x16 in fp8 (keeps entries clear of the
e4m3 subnormal cliff); the x16 cancels exactly: cos/sin tables /16 for
q&k, the AV normalizer leaves out_t = 16*out, and the proj psum is then
256x -- absorbed by shipping resid x256 and LayerNorm scale-invariance.
1/sqrt(HD) folded into Wq; Wo@bv + bo folded into the residual;
attention_mask and bq/bk are structurally zero and not shipped.
"""

import functools
import os
import sys

sys.path.insert(0, "/opt/trn_rl_repo")

import ml_dtypes
import numpy as np

B, L, D, H = 2, 1024, 2048, 16
HD = D // H  # 128
N_CORES = 8
HL = H // N_CORES  # heads per core = 2
DDL = HL * HD  # local head dims = 256
TOK = B * L  # 2048
TS = TOK // N_CORES  # tokens per core = 256
EPS = 1e-12
NDC = D // 128  # 16 chunks along the contraction dim
NDP = NDC // 2  # 8 DoubleRow chunk-pairs

WSC = 16.0  # fp8 weight pre-scale
RSC = WSC * WSC  # proj psum scale (absorbed by resid + LN invariance)
ESHIFT = -2.0  # exp(s-2): keeps exp below the e4m3 max

BF16 = ml_dtypes.bfloat16
F8 = ml_dtypes.float8_e4m3

# set by kernel() after each run; test.py reads it
last_result = None


def _ensure_ntff_hook():
    """Register the axon NTFF profile hook if the image's antenv lacks it."""
    import types

    try:
        from antenv.axon_hooks import get_axon_ntff_profile_hook  # noqa: F401

        return
    except ImportError:
        pass
    try:
        import antenv
        from trn_agent_boot.trn_boot import _ntff_profile_via_ctypes

        hook = _ntff_profile_via_ctypes("/opt/axon/libaxon_pjrt.so")
        mod = types.ModuleType("antenv.axon_hooks")
        mod.get_axon_ntff_profile_hook = lambda: hook
        mod.set_axon_ntff_profile_hook = lambda h: None
        sys.modules["antenv.axon_hooks"] = mod
        antenv.axon_hooks = mod
    except Exception:
        pass


@functools.lru_cache(maxsize=2)
def _build(skip_gb=False):
    from contextlib import ExitStack

    import concourse.tile as tile
    from concourse import bacc, mybir
    from concourse.tile import add_dep_helper

    bf = mybir.dt.bfloat16
    f32 = mybir.dt.float32
    f16 = mybir.dt.float16
    f8 = mybir.dt.float8e4
    Exp = mybir.ActivationFunctionType.Exp
    Sqrt = mybir.ActivationFunctionType.Sqrt
    DR = mybir.MatmulPerfMode.DoubleRow

    nc = bacc.Bacc(
        "TRN2", target_bir_lowering=False, debug=False, num_devices=N_CORES
    )

    # all inputs pre-chunked on host to [partition, chunk, free] so each
    # SBUF partition's data is one contiguous DMA run
    xt_d = nc.dram_tensor("xt", [B, 128, NDC, L], f8, kind="ExternalInput")
    wqkt_d = nc.dram_tensor("wqkt", [128, NDC, 2 * DDL], f8, kind="ExternalInput")
    wvt_d = nc.dram_tensor("wvt", [128, NDC, DDL], f8, kind="ExternalInput")
    wot_d = nc.dram_tensor("wot", [128, NDC, D], f8, kind="ExternalInput")
    cost_d = nc.dram_tensor("cost", [128, L], bf, kind="ExternalInput")
    sint_d = nc.dram_tensor("sint", [128, L], bf, kind="ExternalInput")
    resid_d = nc.dram_tensor("resid", [128, 2, D], f32, kind="ExternalInput")
    gam_d = nc.dram_tensor("gam", [1, D], bf, kind="ExternalInput")
    bet_d = nc.dram_tensor("bet", [1, D], bf, kind="ExternalInput")
    out_d = nc.dram_tensor("out", [TS, D], f32, kind="ExternalOutput")

    with tile.TileContext(nc) as tc:
        with ExitStack() as ctx:
            constp = ctx.enter_context(tc.tile_pool(name="const", bufs=1))
            wqkp = ctx.enter_context(tc.tile_pool(name="wqk", bufs=1))
            wvp = ctx.enter_context(tc.tile_pool(name="wv", bufs=1))
            wop = ctx.enter_context(tc.tile_pool(name="wo", bufs=1))
            xbp = ctx.enter_context(tc.tile_pool(name="xb", bufs=2))
            qkp = ctx.enter_context(tc.tile_pool(name="qk", bufs=8))
            vallp = ctx.enter_context(tc.tile_pool(name="vall", bufs=2))
            expp = ctx.enter_context(tc.tile_pool(name="expt", bufs=8))
            ropetmpp = ctx.enter_context(tc.tile_pool(name="ropetmp", bufs=4))
            ropehalfp = ctx.enter_context(tc.tile_pool(name="ropehalf", bufs=4))
            invbcp = ctx.enter_context(tc.tile_pool(name="invbc", bufs=2))
            sumsp = ctx.enter_context(tc.tile_pool(name="sums_sb", bufs=2))
            outsbp = ctx.enter_context(tc.tile_pool(name="outsb", bufs=2))
            atp = ctx.enter_context(tc.tile_pool(name="at", bufs=2))
            residp = ctx.enter_context(tc.tile_pool(name="resid", bufs=1))
            projfp = ctx.enter_context(tc.tile_pool(name="projf", bufs=2))
            smtp = ctx.enter_context(tc.tile_pool(name="smt", bufs=8))
            psmm = ctx.enter_context(tc.tile_pool(name="ps_mm", bufs=6, space="PSUM"))
            pssums = ctx.enter_context(
                tc.tile_pool(name="ps_sums", bufs=2, space="PSUM")
            )
            dramp = ctx.enter_context(tc.tile_pool(name="dram", bufs=1, space="DRAM"))

            # ---- critical-path loads first: interleave wqk / batch-0 X^T
            # per DoubleRow chunk-pair, 4 DMAs per pair so each pair spans
            # several round-robin hw queues and lands in chase order ----
            wqk_all = wqkp.tile([128, NDC, 2 * DDL], f8, tag="wqk")
            xb = {}
            xb[0] = xbp.tile([128, NDC, L], f8, tag="xb", name="xb0")
            for i in range(NDP):
                for hf in range(2):
                    nc.sync.dma_start(
                        out=wqk_all[:, 2 * i : 2 * i + 2, hf * 256 : hf * 256 + 256],
                        in_=wqkt_d[:, 2 * i : 2 * i + 2, hf * 256 : hf * 256 + 256],
                    )
                    nc.sync.dma_start(
                        out=xb[0][:, 2 * i : 2 * i + 2, hf * 512 : hf * 512 + 512],
                        in_=xt_d[0][:, 2 * i : 2 * i + 2, hf * 512 : hf * 512 + 512],
                    )
            wvt_all = wvp.tile([128, NDC, DDL], f8, tag="wv")
            for c4 in range(4):
                nc.sync.dma_start(
                    out=wvt_all[:, 4 * c4 : 4 * c4 + 4, :],
                    in_=wvt_d[:, 4 * c4 : 4 * c4 + 4, :],
                )
            cos_t = constp.tile([128, L], bf)
            nc.sync.dma_start(out=cos_t, in_=cost_d[:])
            sin_t = constp.tile([128, L], bf)
            nc.sync.dma_start(out=sin_t, in_=sint_d[:])
            xb[1] = xbp.tile([128, NDC, L], f8, tag="xb", name="xb1")
            for c4 in range(4):
                nc.sync.dma_start(
                    out=xb[1][:, 4 * c4 : 4 * c4 + 4, :],
                    in_=xt_d[1][:, 4 * c4 : 4 * c4 + 4, :],
                )

            ones8_t = constp.tile([128, 2, 32], f8)
            nc.vector.memset(ones8_t, 1.0)
            eps_t = constp.tile([128, 1], f32)
            nc.vector.memset(eps_t, EPS)
            eshift_t = constp.tile([128, 1], f32)
            nc.vector.memset(eshift_t, ESHIFT)

            # tiny warm-up AllToAll: absorbs the cross-core sync/launch skew
            # (~20us on the first collective) while the PE is still loading
            warm_sb = constp.tile([1, 128], bf)
            nc.vector.memset(warm_sb, 0.0)
            warm_in = dramp.tile([N_CORES, 16], bf, name="warm_in")
            warm_out = dramp.tile([N_CORES, 16], bf, name="warm_out")
            nc.sync.dma_start(
                out=warm_in[:].rearrange("c t -> (c t)"), in_=warm_sb[0, :]
            )
            nc.gpsimd.collective_compute(
                "AllToAll",
                mybir.AluOpType.bypass,
                replica_groups=[list(range(N_CORES))],
                ins=[warm_in.opt()],
                outs=[warm_out.opt()],
            )
            delayed_loads = []
            if not skip_gb:
                g_bc = constp.tile([128, D], bf)
                i_gbc = nc.gpsimd.dma_start(
                    out=g_bc, in_=gam_d[:].to_broadcast([128, D])
                )
                b_bc = constp.tile([128, D], bf)
                i_bbc = nc.gpsimd.dma_start(
                    out=b_bc, in_=bet_d[:].to_broadcast([128, D])
                )
                delayed_loads += [i_gbc, i_bbc]
            resid_all = residp.tile([128, 2, D], f32, tag="rs")
            i_resid = nc.sync.dma_start(out=resid_all, in_=resid_d[:])
            delayed_loads.append(i_resid)
            wot_sb = wop.tile([128, NDC, D], f8, tag="wo")
            wo_loads = []
            for c4 in range(4):
                i_wo = nc.sync.dma_start(
                    out=wot_sb[:, 4 * c4 : 4 * c4 + 4, :],
                    in_=wot_d[:, 4 * c4 : 4 * c4 + 4, :],
                )
                wo_loads.append(i_wo)
            anchors = []

            # A2A buffers per (batch, head): [src core, dd, tok]
            a2a_in = {}
            a2a_out = {}
            for b in range(B):
                for g in range(HL):
                    a2a_in[(b, g)] = dramp.tile(
                        [N_CORES, HD, 128], f8, name=f"a2ain{b}{g}"
                    )
                    a2a_out[(b, g)] = dramp.tile(
                        [N_CORES, HD, 128], f8, name=f"a2aout{b}{g}"
                    )

            qT = {}  # (b, h) -> roped Q^T (128 d, 1024 tok) bf16
            kT = {}

            def rope_wave(b, ccs, pss):
                """Consume q/k psums [cc][tcs] -> roped bf16 qT/kT tiles.

                ScalarE extracts psum -> bf16 so every DVE operand is 2-byte
                SBUF (2x DVE mode); the lower-half sign of the rotation is
                folded into the sin table (rows 0..63 hold -sin) so the
                combine is one full-width add."""
                for cc in ccs:
                    h = cc % 2
                    dst = qT[(b, h)] if cc < 2 else kT[(b, h)]
                    for tcs in range(2):
                        sl = slice(tcs * 512, (tcs + 1) * 512)
                        ps = pss[(cc, tcs)]
                        tmp = ropetmpp.tile([128, 512], bf, tag="rtmp")
                        nc.scalar.copy(tmp, ps)
                        # partition-swapped copy [x2; x1] (single-input ops
                        # may cross partition bases; tensor_tensor may not)
                        tmps = ropetmpp.tile([128, 512], bf, tag="rtmp")
                        nc.vector.tensor_copy(tmps[0:64, :], tmp[64:128, :])
                        nc.vector.tensor_copy(tmps[64:128, :], tmp[0:64, :])
                        rot = ropehalfp.tile([128, 512], bf, tag="half")
                        nc.vector.tensor_mul(rot, tmp, cos_t[:, sl])
                        rots = ropehalfp.tile([128, 512], bf, tag="half")
                        nc.vector.tensor_mul(rots, tmps, sin_t[:, sl])
                        nc.vector.tensor_add(dst[:, sl], rot, rots)

            for b in range(B):
                for h in range(HL):
                    qT[(b, h)] = qkp.tile([128, L], bf, tag="qk", name=f"qT_{b}_{h}")
                    kT[(b, h)] = qkp.tile([128, L], bf, tag="qk", name=f"kT_{b}_{h}")

                # ---- Q^T / K^T projection (fp8 DoubleRow, i-outer chase) ----
                # wave 0: cc 0,1 (q heads); wave 1: cc 2,3 (k heads)
                for wave in range(2):
                    ccs = (0, 1) if wave == 0 else (2, 3)
                    pss = {}
                    for cc in ccs:
                        for tcs in range(2):
                            pss[(cc, tcs)] = psmm.tile(
                                [128, 512], f32, tag="mm", name=f"qkps{b}{cc}{tcs}"
                            )
                    for i in range(NDP):
                        for cc in ccs:
                            for tcs in range(2):
                                nc.tensor.matmul(
                                    pss[(cc, tcs)],
                                    lhsT=wqk_all[
                                        :, 2 * i : 2 * i + 2, cc * 128 : (cc + 1) * 128
                                    ],
                                    rhs=xb[b][
                                        :, 2 * i : 2 * i + 2, tcs * 512 : (tcs + 1) * 512
                                    ],
                                    start=(i == 0),
                                    stop=(i == NDP - 1),
                                    perf_mode=DR,
                                )
                    rope_wave(b, ccs, pss)

                # ---- V in token-major layout (fp8 DoubleRow, 2 waves) ----
                v_all = vallp.tile([128, 8, DDL], f8, tag="v", name=f"v_{b}")
                for vw in range(2):
                    tc8s = range(vw * 4, vw * 4 + 4)
                    psv = {
                        tc8: psmm.tile([128, DDL], f32, tag="mm", name=f"vps{b}{tc8}")
                        for tc8 in tc8s
                    }
                    for i in range(NDP):
                        for tc8 in tc8s:
                            nc.tensor.matmul(
                                psv[tc8],
                                lhsT=xb[b][
                                    :, 2 * i : 2 * i + 2, tc8 * 128 : (tc8 + 1) * 128
                                ],
                                rhs=wvt_all[:, 2 * i : 2 * i + 2, :],
                                start=(i == 0),
                                stop=(i == NDP - 1),
                                perf_mode=DR,
                            )
                    for tc8 in tc8s:
                        nc.scalar.copy(v_all[:, tc8, :], psv[tc8])

                # ---- attention for the 2 heads of this batch ----
                for h in range(HL):
                    qt = qT[(b, h)]
                    kt = kT[(b, h)]
                    exp_tiles = []
                    for kcp in range(4):
                        et = expp.tile(
                            [128, 2, L], f8, tag="exp", name=f"et_{b}_{h}_{kcp}"
                        )
                        for j in range(2):
                            kc = 2 * kcp + j
                            for qc in range(2):
                                sl = slice(qc * 512, (qc + 1) * 512)
                                ps = psmm.tile([128, 512], f32, tag="mm")
                                i_mm = nc.tensor.matmul(
                                    ps,
                                    lhsT=kt[:, kc * 128 : (kc + 1) * 128],
                                    rhs=qt[:, sl],
                                    start=True,
                                    stop=True,
                                )
                                if kcp == 0 and j == 0 and qc == 0:
                                    anchors.append(i_mm)
                                nc.scalar.activation(
                                    et[:, j, sl], ps, Exp, bias=eshift_t
                                )
                        exp_tiles.append(et)

                    # softmax denominators: fp8-ones DoubleRow matmuls reduce
                    # the 128 k-partitions (and kc pairs) in fp32 psum --
                    # keeps the reduction off the DVE entirely
                    sums_sb = sumsp.tile([1, L], f32, tag="sm")
                    spss = [
                        pssums.tile([32, 512], f32, tag="sums", name=f"spss{i}")
                        for i in range(2)
                    ]
                    for qc in range(2):
                        sl = slice(qc * 512, (qc + 1) * 512)
                        for kcp in range(4):
                            nc.tensor.matmul(
                                spss[qc],
                                lhsT=ones8_t,
                                rhs=exp_tiles[kcp][:, :, sl],
                                start=(kcp == 0),
                                stop=(kcp == 3),
                                perf_mode=DR,
                            )
                        nc.scalar.copy(sums_sb[:, sl], spss[qc][0:1, :])
                    # broadcast sums to all partitions, then 128-lane reciprocal
                    sums_bc = invbcp.tile([128, L], f32, tag="ib", name="sums_bc")
                    nc.gpsimd.partition_broadcast(sums_bc, sums_sb)
                    ib_full = invbcp.tile([128, L], f32, tag="ib", name="ib_full")
                    nc.vector.reciprocal_approx_fast(ib_full, sums_bc)

                    # out^T via tok-major V stationary (fp8 DoubleRow),
                    # normalized; out_t = 16 * out because V is shipped x16
                    out_t = outsbp.tile([128, L], f8, tag="ot")
                    avps = [
                        psmm.tile([128, 512], f32, tag="mm", name=f"avps{i}")
                        for i in range(2)
                    ]
                    for kcp in range(4):
                        for qc in range(2):
                            nc.tensor.matmul(
                                avps[qc],
                                lhsT=v_all[
                                    :, 2 * kcp : 2 * kcp + 2, h * 128 : (h + 1) * 128
                                ],
                                rhs=exp_tiles[kcp][:, :, qc * 512 : (qc + 1) * 512],
                                start=(kcp == 0),
                                stop=(kcp == 3),
                                perf_mode=DR,
                            )
                    for qc in range(2):
                        sl = slice(qc * 512, (qc + 1) * 512)
                        nc.vector.tensor_mul(out_t[:, sl], avps[qc], ib_full[:, sl])
                        # ship this half into the AllToAll input:
                        # tokens qc*512..+512 span destination chunks 4qc..4qc+3
                        s_ap = out_t[:, sl].rearrange("d (c t) -> d c t", c=4)
                        c0 = 4 * qc
                        d_ap = a2a_in[(b, h)][c0 : c0 + 4, :, :]
                        d_ap = d_ap.rearrange("c d t -> d c t")
                        nc.sync.dma_start(out=d_ap, in_=s_ap)

                    # per-head AllToAll so the tail only waits on the last head
                    nc.gpsimd.collective_compute(
                        "AllToAll",
                        mybir.AluOpType.bypass,
                        replica_groups=[list(range(N_CORES))],
                        ins=[a2a_in[(b, h)].opt()],
                        outs=[a2a_out[(b, h)].opt()],
                    )

            for dl in delayed_loads:
                # dl depends on the first scores matmul of batch 0
                add_dep_helper(
                    dl.ins, anchors[0].ins, sync=True, reason="delay-noncritical-load"
                )
            for i_wo in wo_loads:
                add_dep_helper(
                    i_wo.ins, anchors[0].ins, sync=True, reason="delay-wo-load"
                )

            # attn^T (2048 dd, my 128 tokens) per batch, fp8.  Chunk dim is
            # head-major (host reorders Wo chunks to match): chunks 0..7 =
            # head 0 of cores 0..7, chunks 8..15 = head 1.  Chunked loads so
            # the projection chase starts on the first arriving half.
            at = {}
            for b in range(B):
                at[b] = atp.tile([128, NDC, 128], f8, tag="at", name=f"at{b}")
                for g in range(HL):
                    src = a2a_out[(b, g)][:].rearrange("c p t -> p c t")
                    for half in range(2):
                        nc.sync.dma_start(
                            out=at[b][:, g * 8 + 4 * half : g * 8 + 4 * half + 4, :],
                            in_=src[:, 4 * half : 4 * half + 4, :],
                        )

            # ---- output projection (fp8 DoubleRow) + residual + LayerNorm ----
            for tcs in range(TS // 128):
                pf = projfp.tile([128, D], f32, tag="pf")
                psp = [
                    psmm.tile([128, 512], f32, tag="mm", name=f"pps{tcs}{jc}")
                    for jc in range(4)
                ]
                for i in range(NDP):
                    for jc in range(4):
                        nc.tensor.matmul(
                            psp[jc],
                            lhsT=at[tcs][:, 2 * i : 2 * i + 2, :],
                            rhs=wot_sb[
                                :, 2 * i : 2 * i + 2, jc * 512 : (jc + 1) * 512
                            ],
                            start=(i == 0),
                            stop=(i == NDP - 1),
                            perf_mode=DR,
                        )
                stats = smtp.tile([128, 4, 6], f32, tag="st")
                for jc in range(4):
                    # psum = 256 * (out @ Wo); resid shipped x256 -- the
                    # common 256 factor cancels in LayerNorm
                    nc.vector.tensor_add(
                        pf[:, jc * 512 : (jc + 1) * 512],
                        psp[jc],
                        resid_all[:, tcs, jc * 512 : (jc + 1) * 512],
                    )
                    nc.vector.bn_stats(
                        stats[:, jc, :], pf[:, jc * 512 : (jc + 1) * 512]
                    )
                # LayerNorm over D
                mv = smtp.tile([128, 2], f32, tag="mv")
                nc.vector.bn_aggr(mv, stats)
                std = smtp.tile([128, 1], f32, tag="std")
                nc.scalar.activation(std, mv[:, 1:2], Sqrt, bias=eps_t)
                rstd = smtp.tile([128, 1], f32, tag="rstd")
                nc.vector.reciprocal(rstd, std)
                for jc in range(4):
                    sl = slice(jc * 512, (jc + 1) * 512)
                    nc.vector.tensor_scalar(
                        out=pf[:, sl],
                        in0=pf[:, sl],
                        scalar1=mv[:, 0:1],
                        scalar2=rstd,
                        op0=mybir.AluOpType.subtract,
                        op1=mybir.AluOpType.mult,
                    )
                    if not skip_gb:
                        nc.vector.tensor_mul(pf[:, sl], pf[:, sl], g_bc[:, sl])
                        nc.vector.tensor_add(pf[:, sl], pf[:, sl], b_bc[:, sl])
                    nc.sync.dma_start(
                        out=out_d[tcs * 128 : (tcs + 1) * 128, sl], in_=pf[:, sl]
                    )

    nc.compile()
    return nc


def _prep_inputs(
    hidden_state,
    attention_mask,
    freqs,
    Wq,
    bq,
    Wk,
    bk,
    Wv,
    bv,
    Wo,
    bo,
    ln_g,
    ln_b,
):
    hidden_state = np.asarray(hidden_state, dtype=np.float32)
    freqs = np.asarray(freqs, dtype=np.float32)
    Wq = np.asarray(Wq, dtype=np.float32)
    Wk = np.asarray(Wk, dtype=np.float32)
    Wv = np.asarray(Wv, dtype=np.float32)
    Wo = np.asarray(Wo, dtype=np.float32)
    bv = np.asarray(bv, dtype=np.float32)
    bo = np.asarray(bo, dtype=np.float32)
    ln_g = np.asarray(ln_g, dtype=np.float32)
    ln_b = np.asarray(ln_b, dtype=np.float32)

    X = hidden_state.reshape(TOK, D)
    # (B, 128 partition, NDC chunk, L) with contiguous per-partition runs
    xt = np.ascontiguousarray(
        X.reshape(B, L, NDC, 128).transpose(0, 3, 2, 1)
    ).astype(F8)

    # NeoX (even-first) permutation of rows within each head for Wq/Wk, the
    # 1/sqrt(HD) score scale folded into Wq, and the x16 fp8 range scale.
    perm = np.concatenate([np.arange(0, HD, 2), np.arange(1, HD, 2)])
    rows = np.arange(D).reshape(H, HD)[:, perm].reshape(D)
    Wq_p = (Wq * (WSC / np.sqrt(HD)))[rows]
    Wk_p = (Wk * WSC)[rows]

    cosT = np.cos(freqs).T / WSC  # (64, L); /16 cancels the q&k weight scale
    sinT = np.sin(freqs).T / WSC
    cost = np.ascontiguousarray(np.concatenate([cosT, cosT], 0)).astype(BF16)
    # rows 0..63 hold -sin: folds the rotation's lower-half sign into the
    # table so RoPE's combine step is a single full-width add
    sint = np.ascontiguousarray(np.concatenate([-sinT, sinT], 0)).astype(BF16)

    # Wo chunk order is head-major (even global chunks = head 0 of cores
    # 0..7, odd = head 1) to match the per-head A2A arrival order.
    chunk_order = np.concatenate([np.arange(0, NDC, 2), np.arange(1, NDC, 2)])
    wot = np.ascontiguousarray(
        (Wo.T * WSC).reshape(NDC, 128, D)[chunk_order].transpose(1, 0, 2)
    ).astype(F8)  # (128, NDC, D)
    bo_eff = bo + Wo @ bv  # attn rows sum to 1 => bv folds through Wo
    gam = np.ascontiguousarray(ln_g.reshape(1, D)).astype(BF16)
    bet = np.ascontiguousarray(ln_b.reshape(1, D)).astype(BF16)

    skip_gb = bool(np.all(ln_g == 1.0) and np.all(ln_b == 0.0))
    in_maps = []
    for c in range(N_CORES):
        dd = slice(c * DDL, (c + 1) * DDL)
        wqk_c = np.concatenate([Wq_p[dd], Wk_p[dd]], axis=0)  # (512, D)
        wqkt_c = np.ascontiguousarray(
            wqk_c.T.reshape(NDC, 128, 2 * DDL).transpose(1, 0, 2)
        ).astype(F8)
        wvt_c = np.ascontiguousarray(
            (Wv[dd] * WSC).T.reshape(NDC, 128, DDL).transpose(1, 0, 2)
        ).astype(F8)
        tok_rows = np.stack(
            [X[b * L + c * 128 : b * L + (c + 1) * 128] for b in range(B)], axis=1
        )  # (128, B, D)
        resid_c = np.ascontiguousarray(
            (tok_rows + bo_eff[None, None, :]) * RSC
        ).astype(np.float32)
        in_maps.append(
            {
                "xt": xt,
                "wqkt": wqkt_c,
                "wvt": wvt_c,
                "wot": wot,
                "cost": cost,
                "sint": sint,
                "resid": resid_c,
                "gam": gam,
                "bet": bet,
            }
        )
    return skip_gb, in_maps


def _assemble_out(results):
    out = np.empty((B, L, D), dtype=np.float32)
    for c in range(N_CORES):
        r = results[c]["out"]  # (256, D): [b0 tokens; b1 tokens]
        for b in range(B):
            out[b, c * 128 : (c + 1) * 128] = r[b * 128 : (b + 1) * 128]
    return out


def kernel(**inputs):
    global last_result
    _ensure_ntff_hook()
    from concourse.bass_utils import run_bass_kernel_spmd

    skip_gb, in_maps = _prep_inputs(**inputs)
    nc = _build(skip_gb)
    last_result = run_bass_kernel_spmd(
        nc,
        in_maps,
        core_ids=list(range(N_CORES)),
        trace=bool(int(os.environ.get("BASS_TRACE", "0") or "0")),
    )
    return _assemble_out(last_result.results)


# revision 15
# speedup vs baseline: 1.0892x; 1.0892x over previous
"""Trainium2 Bass kernel for nn_DecoderAttention (B=2, L=1024, D=2048, H=16).

Sharding: tensor-parallel over heads (2 heads / core, 8 cores).  v2: all
projection/AV matmuls run in fp8 e4m3 with MatmulPerfMode.DoubleRow
(K=256 per instruction, 2x bf16 throughput); attention scores stay bf16.

Per core:
  1. Q/K projection for its 2 heads (fp8 DoubleRow, i-outer "chase"
     ordering so the PE starts as soon as the first 2 D-chunks land).
     RoPE applied with a host-side NeoX (even/odd) row permutation of
     Wq/Wk so rotation is elementwise on 64-partition halves.
  2. V in token-major layout (fp8 DoubleRow), drained to fp8 SBUF.
  3. Attention per (batch, head): scores^T = K @ Q^T in bf16, exp(s-2)
     on ScalarE straight to fp8 (shift keeps exp under the e4m3 max),
     softmax denominators via f16 tree-add + ones-matmul, out^T via
     fp8 DoubleRow with tok-major V stationary.
  4. Per-(batch,head) AllToAll of the normalized fp8 out^T so core c
     ends with the full 2048 head-dims of its 256 tokens; fp8 DoubleRow
     output projection chases the arriving halves; residual + LayerNorm.

Numerics: Wq/Wk/Wv/Wo shipped x16 in fp8 (keeps entries clear of the
e4m3 subnormal cliff); the x16 cancels exactly: cos/sin tables /16 for
q&k, the AV normalizer leaves out_t = 16*out, and the proj psum is then
256x -- absorbed by shipping resid x256 and LayerNorm scale-invariance.
1/sqrt(HD) folded into Wq; Wo@bv + bo folded into the residual;
attention_mask and bq/bk are structurally zero and not shipped.
"""

import functools
import os
import sys

sys.path.insert(0, "/opt/trn_rl_repo")

import ml_dtypes
import numpy as np

B, L, D, H = 2, 1024, 2048, 16
HD = D // H  # 128
N_CORES = 8
HL = H // N_CORES  # heads per core = 2
DDL = HL * HD  # local head dims = 256
TOK = B * L  # 2048
TS = TOK // N_CORES  # tokens per core = 256
EPS = 1e-12
NDC = D // 128  # 16 chunks along the contraction dim
NDP = NDC // 2  # 8 DoubleRow chunk-pairs

WSC = 16.0  # fp8 weight pre-scale
RSC = WSC * WSC  # proj psum scale (absorbed by resid + LN invariance)
ESHIFT = -2.0  # exp(s-2): keeps exp below the e4m3 max

BF16 = ml_dtypes.bfloat16
F8 = ml_dtypes.float8_e4m3

# set by kernel() after each run; test.py reads it
last_result = None


def _ensure_ntff_hook():
    """Register the axon NTFF profile hook if the image's antenv lacks it."""
    import types

    try:
        from antenv.axon_hooks import get_axon_ntff_profile_hook  # noqa: F401

        return
    except ImportError:
        pass
    try:
        import antenv
        from trn_agent_boot.trn_boot import _ntff_profile_via_ctypes

        hook = _ntff_profile_via_ctypes("/opt/axon/libaxon_pjrt.so")
        mod = types.ModuleType("antenv.axon_hooks")
        mod.get_axon_ntff_profile_hook = lambda: hook
        mod.set_axon_ntff_profile_hook = lambda h: None
        sys.modules["antenv.axon_hooks"] = mod
        antenv.axon_hooks = mod
    except Exception:
        pass


@functools.lru_cache(maxsize=2)
def _build(skip_gb=False):
    from contextlib import ExitStack

    import concourse.tile as tile
    from concourse import bacc, mybir
    from concourse.tile import add_dep_helper

    bf = mybir.dt.bfloat16
    f32 = mybir.dt.float32
    f16 = mybir.dt.float16
    f8 = mybir.dt.float8e4
    Exp = mybir.ActivationFunctionType.Exp
    Sqrt = mybir.ActivationFunctionType.Sqrt
    DR = mybir.MatmulPerfMode.DoubleRow

    nc = bacc.Bacc(
        "TRN2", target_bir_lowering=False, debug=False, num_devices=N_CORES
    )

    # all inputs pre-chunked on host to [partition, chunk, free] so each
    # SBUF partition's data is one contiguous DMA run
    xt_d = nc.dram_tensor("xt", [B, 128, NDC, L], f8, kind="ExternalInput")
    wqkt_d = nc.dram_tensor("wqkt", [128, NDC, 2 * DDL], f8, kind="ExternalInput")
    wvt_d = nc.dram_tensor("wvt", [128, NDC, DDL], f8, kind="ExternalInput")
    wot_d = nc.dram_tensor("wot", [128, NDC, D], f8, kind="ExternalInput")
    cost_d = nc.dram_tensor("cost", [128, L], bf, kind="ExternalInput")
    sint_d = nc.dram_tensor("sint", [128, L], bf, kind="ExternalInput")
    resid_d = nc.dram_tensor("resid", [128, 2, D], f32, kind="ExternalInput")
    gam_d = nc.dram_tensor("gam", [1, D], bf, kind="ExternalInput")
    bet_d = nc.dram_tensor("bet", [1, D], bf, kind="ExternalInput")
    out_d = nc.dram_tensor("out", [TS, D], f32, kind="ExternalOutput")

    with tile.TileContext(nc) as tc:
        with ExitStack() as ctx:
            constp = ctx.enter_context(tc.tile_pool(name="const", bufs=1))
            wqkp = ctx.enter_context(tc.tile_pool(name="wqk", bufs=1))
            wvp = ctx.enter_context(tc.tile_pool(name="wv", bufs=1))
            wop = ctx.enter_context(tc.tile_pool(name="wo", bufs=1))
            xbp = ctx.enter_context(tc.tile_pool(name="xb", bufs=2))
            qkp = ctx.enter_context(tc.tile_pool(name="qk", bufs=8))
            vallp = ctx.enter_context(tc.tile_pool(name="vall", bufs=2))
            expp = ctx.enter_context(tc.tile_pool(name="expt", bufs=8))
            ropetmpp = ctx.enter_context(tc.tile_pool(name="ropetmp", bufs=4))
            ropehalfp = ctx.enter_context(tc.tile_pool(name="ropehalf", bufs=4))
            invbcp = ctx.enter_context(tc.tile_pool(name="invbc", bufs=2))
            sumsp = ctx.enter_context(tc.tile_pool(name="sums_sb", bufs=2))
            outsbp = ctx.enter_context(tc.tile_pool(name="outsb", bufs=2))
            atp = ctx.enter_context(tc.tile_pool(name="at", bufs=2))
            residp = ctx.enter_context(tc.tile_pool(name="resid", bufs=1))
            projfp = ctx.enter_context(tc.tile_pool(name="projf", bufs=2))
            smtp = ctx.enter_context(tc.tile_pool(name="smt", bufs=8))
            psmm = ctx.enter_context(tc.tile_pool(name="ps_mm", bufs=6, space="PSUM"))
            pssums = ctx.enter_context(
                tc.tile_pool(name="ps_sums", bufs=2, space="PSUM")
            )
            dramp = ctx.enter_context(tc.tile_pool(name="dram", bufs=1, space="DRAM"))

            # warm-up A2A input must be the very first DMA issued or it queues
            # behind the bulk loads and head-of-line-blocks the CC pipeline
            warm_sb = constp.tile([1, 128], bf)
            nc.vector.memset(warm_sb, 0.0)
            warm_in = dramp.tile([N_CORES, 16], bf, name="warm_in")
            warm_out = dramp.tile([N_CORES, 16], bf, name="warm_out")
            nc.sync.dma_start(
                out=warm_in[:].rearrange("c t -> (c t)"), in_=warm_sb[0, :]
            )
            nc.gpsimd.collective_compute(
                "AllToAll",
                mybir.AluOpType.bypass,
                replica_groups=[list(range(N_CORES))],
                ins=[warm_in.opt()],
                outs=[warm_out.opt()],
            )

            # ---- critical-path loads first: interleave wqk / batch-0 X^T
            # per DoubleRow chunk-pair, split on the partition dim (keeps the
            # 2KB contiguous runs) so each pair spans several hw queues and
            # lands in chase order ----
            wqk_all = wqkp.tile([128, NDC, 2 * DDL], f8, tag="wqk")
            xb = {}
            xb[0] = xbp.tile([128, NDC, L], f8, tag="xb", name="xb0")
            for i in range(NDP):
                for hf in range(2):
                    pp = slice(hf * 64, hf * 64 + 64)
                    nc.sync.dma_start(
                        out=wqk_all[pp, 2 * i : 2 * i + 2, :],
                        in_=wqkt_d[pp, 2 * i : 2 * i + 2, :],
                    )
                    nc.sync.dma_start(
                        out=xb[0][pp, 2 * i : 2 * i + 2, :],
                        in_=xt_d[0][pp, 2 * i : 2 * i + 2, :],
                    )
            wvt_all = wvp.tile([128, NDC, DDL], f8, tag="wv")
            for c4 in range(4):
                nc.sync.dma_start(
                    out=wvt_all[:, 4 * c4 : 4 * c4 + 4, :],
                    in_=wvt_d[:, 4 * c4 : 4 * c4 + 4, :],
                )
            cos_t = constp.tile([128, L], bf)
            nc.sync.dma_start(out=cos_t, in_=cost_d[:])
            sin_t = constp.tile([128, L], bf)
            nc.sync.dma_start(out=sin_t, in_=sint_d[:])
            xb[1] = xbp.tile([128, NDC, L], f8, tag="xb", name="xb1")
            for c4 in range(4):
                nc.sync.dma_start(
                    out=xb[1][:, 4 * c4 : 4 * c4 + 4, :],
                    in_=xt_d[1][:, 4 * c4 : 4 * c4 + 4, :],
                )

            ones8_t = constp.tile([128, 2, 32], f8)
            nc.vector.memset(ones8_t, 1.0)
            eps_t = constp.tile([128, 1], f32)
            nc.vector.memset(eps_t, EPS)
            eshift_t = constp.tile([128, 1], f32)
            nc.vector.memset(eshift_t, ESHIFT)
            delayed_loads = []
            if not skip_gb:
                g_bc = constp.tile([128, D], bf)
                i_gbc = nc.gpsimd.dma_start(
                    out=g_bc, in_=gam_d[:].to_broadcast([128, D])
                )
                b_bc = constp.tile([128, D], bf)
                i_bbc = nc.gpsimd.dma_start(
                    out=b_bc, in_=bet_d[:].to_broadcast([128, D])
                )
                delayed_loads += [i_gbc, i_bbc]
            resid_all = residp.tile([128, 2, D], f32, tag="rs")
            i_resid = nc.sync.dma_start(out=resid_all, in_=resid_d[:])
            delayed_loads.append(i_resid)
            wot_sb = wop.tile([128, NDC, D], f8, tag="wo")
            wo_loads = []
            for c4 in range(4):
                i_wo = nc.sync.dma_start(
                    out=wot_sb[:, 4 * c4 : 4 * c4 + 4, :],
                    in_=wot_d[:, 4 * c4 : 4 * c4 + 4, :],
                )
                wo_loads.append(i_wo)
            anchors = []

            # A2A buffers per (batch, head): [src core, dd, tok]
            a2a_in = {}
            a2a_out = {}
            for b in range(B):
                for g in range(HL):
                    a2a_in[(b, g)] = dramp.tile(
                        [N_CORES, HD, 128], f8, name=f"a2ain{b}{g}"
                    )
                    a2a_out[(b, g)] = dramp.tile(
                        [N_CORES, HD, 128], f8, name=f"a2aout{b}{g}"
                    )

            qT = {}  # (b, h) -> roped Q^T (128 d, 1024 tok) bf16
            kT = {}

            def rope_wave(b, ccs, pss):
                """Consume q/k psums [cc][tcs] -> roped bf16 qT/kT tiles.

                ScalarE extracts psum -> bf16 so every DVE operand is 2-byte
                SBUF (2x DVE mode); the lower-half sign of the rotation is
                folded into the sin table (rows 0..63 hold -sin) so the
                combine is one full-width add."""
                for cc in ccs:
                    h = cc % 2
                    dst = qT[(b, h)] if cc < 2 else kT[(b, h)]
                    for tcs in range(2):
                        sl = slice(tcs * 512, (tcs + 1) * 512)
                        ps = pss[(cc, tcs)]
                        tmp = ropetmpp.tile([128, 512], bf, tag="rtmp")
                        nc.scalar.copy(tmp, ps)
                        # partition-swapped copy [x2; x1] (single-input ops
                        # may cross partition bases; tensor_tensor may not)
                        tmps = ropetmpp.tile([128, 512], bf, tag="rtmp")
                        nc.vector.tensor_copy(tmps[0:64, :], tmp[64:128, :])
                        nc.vector.tensor_copy(tmps[64:128, :], tmp[0:64, :])
                        rot = ropehalfp.tile([128, 512], bf, tag="half")
                        nc.vector.tensor_mul(rot, tmp, cos_t[:, sl])
                        rots = ropehalfp.tile([128, 512], bf, tag="half")
                        nc.vector.tensor_mul(rots, tmps, sin_t[:, sl])
                        nc.vector.tensor_add(dst[:, sl], rot, rots)

            for b in range(B):
                for h in range(HL):
                    qT[(b, h)] = qkp.tile([128, L], bf, tag="qk", name=f"qT_{b}_{h}")
                    kT[(b, h)] = qkp.tile([128, L], bf, tag="qk", name=f"kT_{b}_{h}")

                # ---- Q^T / K^T projection (fp8 DoubleRow, i-outer chase) ----
                # wave 0: cc 0,1 (q heads); wave 1: cc 2,3 (k heads)
                for wave in range(2):
                    ccs = (0, 1) if wave == 0 else (2, 3)
                    pss = {}
                    for cc in ccs:
                        for tcs in range(2):
                            pss[(cc, tcs)] = psmm.tile(
                                [128, 512], f32, tag="mm", name=f"qkps{b}{cc}{tcs}"
                            )
                    for i in range(NDP):
                        for cc in ccs:
                            for tcs in range(2):
                                nc.tensor.matmul(
                                    pss[(cc, tcs)],
                                    lhsT=wqk_all[
                                        :, 2 * i : 2 * i + 2, cc * 128 : (cc + 1) * 128
                                    ],
                                    rhs=xb[b][
                                        :, 2 * i : 2 * i + 2, tcs * 512 : (tcs + 1) * 512
                                    ],
                                    start=(i == 0),
                                    stop=(i == NDP - 1),
                                    perf_mode=DR,
                                )
                    rope_wave(b, ccs, pss)

                # ---- V in token-major layout (fp8 DoubleRow, 2 waves) ----
                v_all = vallp.tile([128, 8, DDL], f8, tag="v", name=f"v_{b}")
                for vw in range(2):
                    tc8s = range(vw * 4, vw * 4 + 4)
                    psv = {
                        tc8: psmm.tile([128, DDL], f32, tag="mm", name=f"vps{b}{tc8}")
                        for tc8 in tc8s
                    }
                    for i in range(NDP):
                        for tc8 in tc8s:
                            nc.tensor.matmul(
                                psv[tc8],
                                lhsT=xb[b][
                                    :, 2 * i : 2 * i + 2, tc8 * 128 : (tc8 + 1) * 128
                                ],
                                rhs=wvt_all[:, 2 * i : 2 * i + 2, :],
                                start=(i == 0),
                                stop=(i == NDP - 1),
                                perf_mode=DR,
                            )
                    for tc8 in tc8s:
                        nc.scalar.copy(v_all[:, tc8, :], psv[tc8])

                # ---- attention for the 2 heads of this batch ----
                for h in range(HL):
                    qt = qT[(b, h)]
                    kt = kT[(b, h)]
                    exp_tiles = []
                    for kcp in range(4):
                        et = expp.tile(
                            [128, 2, L], f8, tag="exp", name=f"et_{b}_{h}_{kcp}"
                        )
                        for j in range(2):
                            kc = 2 * kcp + j
                            for qc in range(2):
                                sl = slice(qc * 512, (qc + 1) * 512)
                                ps = psmm.tile([128, 512], f32, tag="mm")
                                i_mm = nc.tensor.matmul(
                                    ps,
                                    lhsT=kt[:, kc * 128 : (kc + 1) * 128],
                                    rhs=qt[:, sl],
                                    start=True,
                                    stop=True,
                                )
                                if kcp == 0 and j == 0 and qc == 0:
                                    anchors.append(i_mm)
                                nc.scalar.activation(
                                    et[:, j, sl], ps, Exp, bias=eshift_t
                                )
                        exp_tiles.append(et)

                    # softmax denominators: fp8-ones DoubleRow matmuls reduce
                    # the 128 k-partitions (and kc pairs) in fp32 psum --
                    # keeps the reduction off the DVE entirely
                    sums_sb = sumsp.tile([1, L], f32, tag="sm")
                    spss = [
                        pssums.tile([32, 512], f32, tag="sums", name=f"spss{i}")
                        for i in range(2)
                    ]
                    for qc in range(2):
                        sl = slice(qc * 512, (qc + 1) * 512)
                        for kcp in range(4):
                            nc.tensor.matmul(
                                spss[qc],
                                lhsT=ones8_t,
                                rhs=exp_tiles[kcp][:, :, sl],
                                start=(kcp == 0),
                                stop=(kcp == 3),
                                perf_mode=DR,
                            )
                        nc.scalar.copy(sums_sb[:, sl], spss[qc][0:1, :])
                    # broadcast sums to all partitions, then 128-lane reciprocal
                    sums_bc = invbcp.tile([128, L], f32, tag="ib", name="sums_bc")
                    nc.gpsimd.partition_broadcast(sums_bc, sums_sb)
                    ib_full = invbcp.tile([128, L], f32, tag="ib", name="ib_full")
                    nc.vector.reciprocal_approx_fast(ib_full, sums_bc)

                    # out^T via tok-major V stationary (fp8 DoubleRow),
                    # normalized; out_t = 16 * out because V is shipped x16
                    out_t = outsbp.tile([128, L], f8, tag="ot")
                    avps = [
                        psmm.tile([128, 512], f32, tag="mm", name=f"avps{i}")
                        for i in range(2)
                    ]
                    for kcp in range(4):
                        for qc in range(2):
                            nc.tensor.matmul(
                                avps[qc],
                                lhsT=v_all[
                                    :, 2 * kcp : 2 * kcp + 2, h * 128 : (h + 1) * 128
                                ],
                                rhs=exp_tiles[kcp][:, :, qc * 512 : (qc + 1) * 512],
                                start=(kcp == 0),
                                stop=(kcp == 3),
                                perf_mode=DR,
                            )
                    for qc in range(2):
                        sl = slice(qc * 512, (qc + 1) * 512)
                        nc.vector.tensor_mul(out_t[:, sl], avps[qc], ib_full[:, sl])
                        # ship this half into the AllToAll input:
                        # tokens qc*512..+512 span destination chunks 4qc..4qc+3
                        s_ap = out_t[:, sl].rearrange("d (c t) -> d c t", c=4)
                        c0 = 4 * qc
                        d_ap = a2a_in[(b, h)][c0 : c0 + 4, :, :]
                        d_ap = d_ap.rearrange("c d t -> d c t")
                        nc.sync.dma_start(out=d_ap, in_=s_ap)

                    # per-head AllToAll so the tail only waits on the last head
                    nc.gpsimd.collective_compute(
                        "AllToAll",
                        mybir.AluOpType.bypass,
                        replica_groups=[list(range(N_CORES))],
                        ins=[a2a_in[(b, h)].opt()],
                        outs=[a2a_out[(b, h)].opt()],
                    )

            for dl in delayed_loads:
                # dl depends on the first scores matmul of batch 0
                add_dep_helper(
                    dl.ins, anchors[0].ins, sync=True, reason="delay-noncritical-load"
                )
            for i_wo in wo_loads:
                # behind b0h1's first scores matmul: clears the DMA queues for
                # xb1 during the 30..50us window
                add_dep_helper(
                    i_wo.ins, anchors[1].ins, sync=True, reason="delay-wo-load"
                )

            # attn^T (2048 dd, my 128 tokens) per batch, fp8.  Chunk dim is
            # head-major (host reorders Wo chunks to match): chunks 0..7 =
            # head 0 of cores 0..7, chunks 8..15 = head 1.  Chunked loads so
            # the projection chase starts on the first arriving half.
            at = {}
            for b in range(B):
                at[b] = atp.tile([128, NDC, 128], f8, tag="at", name=f"at{b}")
                for g in range(HL):
                    src = a2a_out[(b, g)][:].rearrange("c p t -> p c t")
                    for half in range(2):
                        nc.sync.dma_start(
                            out=at[b][:, g * 8 + 4 * half : g * 8 + 4 * half + 4, :],
                            in_=src[:, 4 * half : 4 * half + 4, :],
                        )

            # ---- output projection (fp8 DoubleRow) + residual + LayerNorm ----
            for tcs in range(TS // 128):
                pf = projfp.tile([128, D], f32, tag="pf")
                psp = [
                    psmm.tile([128, 512], f32, tag="mm", name=f"pps{tcs}{jc}")
                    for jc in range(4)
                ]
                for i in range(NDP):
                    for jc in range(4):
                        nc.tensor.matmul(
                            psp[jc],
                            lhsT=at[tcs][:, 2 * i : 2 * i + 2, :],
                            rhs=wot_sb[
                                :, 2 * i : 2 * i + 2, jc * 512 : (jc + 1) * 512
                            ],
                            start=(i == 0),
                            stop=(i == NDP - 1),
                            perf_mode=DR,
                        )
                stats = smtp.tile([128, 4, 6], f32, tag="st")
                for jc in range(4):
                    # psum = 256 * (out @ Wo); resid shipped x256 -- the
                    # common 256 factor cancels in LayerNorm
                    nc.vector.tensor_add(
                        pf[:, jc * 512 : (jc + 1) * 512],
                        psp[jc],
                        resid_all[:, tcs, jc * 512 : (jc + 1) * 512],
                    )
                    nc.vector.bn_stats(
                        stats[:, jc, :], pf[:, jc * 512 : (jc + 1) * 512]
                    )
                # LayerNorm over D
                mv = smtp.tile([128, 2], f32, tag="mv")
                nc.vector.bn_aggr(mv, stats)
                std = smtp.tile([128, 1], f32, tag="std")
                nc.scalar.activation(std, mv[:, 1:2], Sqrt, bias=eps_t)
                rstd = smtp.tile([128, 1], f32, tag="rstd")
                nc.vector.reciprocal(rstd, std)
                for jc in range(4):
                    sl = slice(jc * 512, (jc + 1) * 512)
                    nc.vector.tensor_scalar(
                        out=pf[:, sl],
                        in0=pf[:, sl],
                        scalar1=mv[:, 0:1],
                        scalar2=rstd,
                        op0=mybir.AluOpType.subtract,
                        op1=mybir.AluOpType.mult,
                    )
                    if not skip_gb:
                        nc.vector.tensor_mul(pf[:, sl], pf[:, sl], g_bc[:, sl])
                        nc.vector.tensor_add(pf[:, sl], pf[:, sl], b_bc[:, sl])
                    nc.sync.dma_start(
                        out=out_d[tcs * 128 : (tcs + 1) * 128, sl], in_=pf[:, sl]
                    )

    nc.compile()
    return nc


def _prep_inputs(
    hidden_state,
    attention_mask,
    freqs,
    Wq,
    bq,
    Wk,
    bk,
    Wv,
    bv,
    Wo,
    bo,
    ln_g,
    ln_b,
):
    hidden_state = np.asarray(hidden_state, dtype=np.float32)
    freqs = np.asarray(freqs, dtype=np.float32)
    Wq = np.asarray(Wq, dtype=np.float32)
    Wk = np.asarray(Wk, dtype=np.float32)
    Wv = np.asarray(Wv, dtype=np.float32)
    Wo = np.asarray(Wo, dtype=np.float32)
    bv = np.asarray(bv, dtype=np.float32)
    bo = np.asarray(bo, dtype=np.float32)
    ln_g = np.asarray(ln_g, dtype=np.float32)
    ln_b = np.asarray(ln_b, dtype=np.float32)

    X = hidden_state.reshape(TOK, D)
    # (B, 128 partition, NDC chunk, L) with contiguous per-partition runs
    xt = np.ascontiguousarray(
        X.reshape(B, L, NDC, 128).transpose(0, 3, 2, 1)
    ).astype(F8)

    # NeoX (even-first) permutation of rows within each head for Wq/Wk, the
    # 1/sqrt(HD) score scale folded into Wq, and the x16 fp8 range scale.
    perm = np.concatenate([np.arange(0, HD, 2), np.arange(1, HD, 2)])
    rows = np.arange(D).reshape(H, HD)[:, perm].reshape(D)
    Wq_p = (Wq * (WSC / np.sqrt(HD)))[rows]
    Wk_p = (Wk * WSC)[rows]

    cosT = np.cos(freqs).T / WSC  # (64, L); /16 cancels the q&k weight scale
    sinT = np.sin(freqs).T / WSC
    cost = np.ascontiguousarray(np.concatenate([cosT, cosT], 0)).astype(BF16)
    # rows 0..63 hold -sin: folds the rotation's lower-half sign into the
    # table so RoPE's combine step is a single full-width add
    sint = np.ascontiguousarray(np.concatenate([-sinT, sinT], 0)).astype(BF16)

    # Wo chunk order is head-major (even global chunks = head 0 of cores
    # 0..7, odd = head 1) to match the per-head A2A arrival order.
    chunk_order = np.concatenate([np.arange(0, NDC, 2), np.arange(1, NDC, 2)])
    wot = np.ascontiguousarray(
        (Wo.T * WSC).reshape(NDC, 128, D)[chunk_order].transpose(1, 0, 2)
    ).astype(F8)  # (128, NDC, D)
    bo_eff = bo + Wo @ bv  # attn rows sum to 1 => bv folds through Wo
    gam = np.ascontiguousarray(ln_g.reshape(1, D)).astype(BF16)
    bet = np.ascontiguousarray(ln_b.reshape(1, D)).astype(BF16)

    skip_gb = bool(np.all(ln_g == 1.0) and np.all(ln_b == 0.0))
    in_maps = []
    for c in range(N_CORES):
        dd = slice(c * DDL, (c + 1) * DDL)
        wqk_c = np.concatenate([Wq_p[dd], Wk_p[dd]], axis=0)  # (512, D)
        wqkt_c = np.ascontiguousarray(
            wqk_c.T.reshape(NDC, 128, 2 * DDL).transpose(1, 0, 2)
        ).astype(F8)
        wvt_c = np.ascontiguousarray(
            (Wv[dd] * WSC).T.reshape(NDC, 128, DDL).transpose(1, 0, 2)
        ).astype(F8)
        tok_rows = np.stack(
            [X[b * L + c * 128 : b * L + (c + 1) * 128] for b in range(B)], axis=1
        )  # (128, B, D)
        resid_c = np.ascontiguousarray(
            (tok_rows + bo_eff[None, None, :]) * RSC
        ).astype(np.float32)
        in_maps.append(
            {
                "xt": xt,
                "wqkt": wqkt_c,
                "wvt": wvt_c,
                "wot": wot,
                "cost": cost,
                "sint": sint,
                "resid": resid_c,
                "gam": gam,
                "bet": bet,
            }
        )
    return skip_gb, in_maps


def _assemble_out(results):
    out = np.empty((B, L, D), dtype=np.float32)
    for c in range(N_CORES):
        r = results[c]["out"]  # (256, D): [b0 tokens; b1 tokens]
        for b in range(B):
            out[b, c * 128 : (c + 1) * 128] = r[b * 128 : (b + 1) * 128]
    return out


def kernel(**inputs):
    global last_result
    _ensure_ntff_hook()
    from concourse.bass_utils import run_bass_kernel_spmd

    skip_gb, in_maps = _prep_inputs(**inputs)
    nc = _build(skip_gb)
    last_result = run_bass_kernel_spmd(
        nc,
        in_maps,
        core_ids=list(range(N_CORES)),
        trace=bool(int(os.environ.get("BASS_TRACE", "0") or "0")),
    )
    return _assemble_out(last_result.results)
